# revision 5
# baseline (speedup 1.0000x reference)
"""EvULoss Trainium2 kernel.

Math (see the EvU loss definition):
    evidence = exp(logits); alpha = evidence + 1
    max_alpha  = exp(rowmax(logits)) + 1
    predictions== target  <=>  logits[n, target[n]] == rowmax(logits[n])   (tie-free)
    sum_alpha  = rowsum(exp(logits)) + C
    unc        = C / sum_alpha
Everything downstream of {rowmax, rowsumexp} is O(N) and runs at gather
time on the host; the O(N*C) streaming pass (DMA + exp + reductions) runs
on 8 NeuronCores, data-parallel over N.

Per core: shard [8192, 1000] f32, viewed as 16 blocks of [128p, 4, 1000].
  - one 2 MB DMA per block (HWDGE)
  - VectorE: reduce_max over free dims -> rowmax column slice [128, 4]
  - ScalarE: exp with accum_out       -> rowsumexp columns    [128, 1] x4
Outputs per core: rowmax[128, 64], sumexp[128, 64] with n = col*128 + p.
"""

from contextlib import ExitStack

import numpy as np

import concourse.bacc as bacc
import concourse.tile as tile
from concourse import mybir
from concourse.bass_utils import run_bass_kernel_spmd

N_CORES = 8
N, C = 65536, 1000
NSHARD = N // N_CORES          # 8192 rows per core
P = 128                        # SBUF partitions
BLK = 4                        # 128-row blocks per DMA: [128, 4, 1000] f32 = 2 MB
NT = NSHARD // (P * BLK)       # 16 outer iterations
TCOLS = NSHARD // P            # 64 stat columns
EPS = 1e-10
BETA = 1.0

_NC_CACHE = None


def _build_bass():
    nc = bacc.Bacc("TRN2", target_bir_lowering=False)
    x = nc.dram_tensor("x", [NSHARD, C], mybir.dt.float32, kind="ExternalInput")
    rowmax = nc.dram_tensor(
        "rowmax", [P, TCOLS], mybir.dt.float32, kind="ExternalOutput"
    )
    sumexp = nc.dram_tensor(
        "sumexp", [P, TCOLS], mybir.dt.float32, kind="ExternalOutput"
    )

    # shard row n = t*(BLK*P) + i*P + p  ->  xv[t][p, i, c]
    xv = x.ap().rearrange("(t i p) c -> t p i c", i=BLK, p=P)
    NEG = float(np.finfo(np.float32).min)

    with tile.TileContext(nc) as tc:
        with ExitStack() as ctx:
            xin = ctx.enter_context(tc.tile_pool(name="xin", bufs=4))
            expp = ctx.enter_context(tc.tile_pool(name="expp", bufs=2))
            scrp = ctx.enter_context(tc.tile_pool(name="scrp", bufs=1))
            stats = ctx.enter_context(tc.tile_pool(name="stats", bufs=1))

            smax = stats.tile([P, TCOLS], mybir.dt.float32)
            ssum = stats.tile([P, TCOLS], mybir.dt.float32)
            # garbage sinks for the elementwise outputs of ts-accum ops
            scr = scrp.tile([P, C], mybir.dt.float32)   # DVE-owned
            ascr = scrp.tile([P, C], mybir.dt.float32)  # ACT-owned

            for t in range(NT):
                xt = xin.tile([P, BLK, C], mybir.dt.float32)
                nc.sync.dma_start(out=xt, in_=xv[t])

                # row-max on DVE: tensor_scalar(max) + max-accumulate, 2x fp32
                for i in range(BLK):
                    col = t * BLK + i
                    nc.vector.tensor_scalar(
                        out=scr,
                        in0=xt[:, i, :],
                        scalar1=NEG,
                        scalar2=None,
                        op0=mybir.AluOpType.max,
                        op1=mybir.AluOpType.max,
                        accum_out=smax[:, col : col + 1],
                    )

                if t % 3 == 0:
                    # ACT computes exp AND the row-sum (accumulator path);
                    # keeps DVE free on these tiles.
                    for i in range(BLK):
                        col = t * BLK + i
                        nc.scalar.activation(
                            ascr,
                            xt[:, i, :],
                            mybir.ActivationFunctionType.Exp,
                            accum_out=ssum[:, col : col + 1],
                        )
                else:
                    # one big exp on ACT; row-sums on DVE via ts-add-accum
                    et = expp.tile([P, BLK, C], mybir.dt.float32)
                    nc.scalar.activation(
                        et, xt, mybir.ActivationFunctionType.Exp
                    )
                    for i in range(BLK):
                        col = t * BLK + i
                        nc.vector.tensor_scalar(
                            out=scr,
                            in0=et[:, i, :],
                            scalar1=1.0,
                            scalar2=None,
                            op0=mybir.AluOpType.mult,
                            op1=mybir.AluOpType.add,
                            accum_out=ssum[:, col : col + 1],
                        )

            nc.sync.dma_start(out=rowmax.ap(), in_=smax)
            nc.sync.dma_start(out=sumexp.ap(), in_=ssum)
    nc.compile()
    return nc


def _run_spmd(output_f32, trace=False, **kwargs):
    """Run the streaming pass on 8 cores. Returns (rowmax[N], sumexp[N], results)."""
    global _NC_CACHE
    if _NC_CACHE is None:
        _NC_CACHE = _build_bass()
    nc = _NC_CACHE
    in_maps = [
        {"x": output_f32[c * NSHARD : (c + 1) * NSHARD]} for c in range(N_CORES)
    ]
    res = run_bass_kernel_spmd(
        nc, in_maps, core_ids=list(range(N_CORES)), trace=trace, **kwargs
    )
    # out[p, col] -> shard row col*128 + p
    rowmax = np.concatenate(
        [r["rowmax"].T.reshape(-1) for r in res.results]
    )
    sumexp = np.concatenate(
        [r["sumexp"].T.reshape(-1) for r in res.results]
    )
    return rowmax, sumexp, res


def kernel(output, target, optimal_uncertainty_threshold, num_classes):
    output = np.ascontiguousarray(np.asarray(output), dtype=np.float32)
    target = np.asarray(target).astype(np.int64)
    th = float(np.asarray(optimal_uncertainty_threshold).reshape(-1)[0])
    c = float(int(num_classes))

    rowmax, sumexp, _ = _run_spmd(output)

    max_alpha = np.exp(rowmax.astype(np.float64)) + 1.0
    sum_alpha = sumexp.astype(np.float64) + c
    unc = c / sum_alpha

    umin = unc.min()
    umax = unc.max()
    unc_th = umin + th * (umax - umin)

    picked = output[np.arange(N), target]
    correct = picked == rowmax
    certain = unc <= unc_th
    t = np.tanh(unc)

    n_ac = np.sum(np.where(correct & certain, max_alpha * (1.0 - t), 0.0))
    n_au = np.sum(np.where(correct & ~certain, max_alpha * t, 0.0))
    n_ic = np.sum(np.where(~correct & certain, (1.0 - max_alpha) * (1.0 - t), 0.0))
    n_iu = np.sum(np.where(~correct & ~certain, (1.0 - max_alpha) * t, 0.0))

    evu = (n_ac + n_iu) / (n_ac + n_au + n_ic + n_iu + EPS)
    loss = -BETA * np.log(evu + EPS)
    return np.array([loss], dtype=np.float32)


# revision 15
# speedup vs baseline: 1.4135x; 1.4135x over previous
"""EvULoss Trainium2 kernel.

Math (see the EvU loss definition):
    evidence = exp(logits); alpha = evidence + 1
    max_alpha  = exp(rowmax(logits)) + 1
    predictions== target  <=>  logits[n, target[n]] == rowmax(logits[n])   (tie-free)
    sum_alpha  = rowsum(exp(logits)) + C
    unc        = C / sum_alpha
Everything downstream of {rowmax, rowsumexp} is O(N) and runs at gather
time on the host; the O(N*C) streaming pass (DMA + exp + reductions) runs
on 8 NeuronCores, data-parallel over N.

Per core: shard [8192, 1000] f32 as 64 blocks of [128 rows, 1000], DMA'd in
ramped tiles ([1,1,2,4]+[4]*13+[2,1,1] blocks; 0.5-2 MB HWDGE transfers).
  - VectorE: reduce_max over free dims -> rowmax columns (one op per tile)
  - ScalarE: exp with accum_out       -> rowsumexp column per 128-row block
    (the fused accumulator read pipelines with the next ACTIVATE; ScalarE is
    the single pacing engine at ~92% busy — measured faster than splitting
    work across engines, which adds coupling stalls)
Outputs per core: rowmax[128, 64], sumexp[128, 64] with shard row = col*128+p.
"""

from contextlib import ExitStack

import numpy as np

import concourse.bacc as bacc
import concourse.tile as tile
from concourse import mybir
from concourse.bass_utils import run_bass_kernel_spmd

N_CORES = 8
N, C = 65536, 1000
NSHARD = N // N_CORES          # 8192 rows per core
P = 128                        # SBUF partitions
BLK = 4                        # 128-row blocks per DMA: [128, 4, 1000] f32 = 2 MB
NT = NSHARD // (P * BLK)       # 16 outer iterations
TCOLS = NSHARD // P            # 64 stat columns
EPS = 1e-10
BETA = 1.0

_NC_CACHE = None


def _build_bass():
    nc = bacc.Bacc("TRN2", target_bir_lowering=False)
    x = nc.dram_tensor("x", [NSHARD, C], mybir.dt.float32, kind="ExternalInput")
    rowmax = nc.dram_tensor(
        "rowmax", [P, TCOLS], mybir.dt.float32, kind="ExternalOutput"
    )
    sumexp = nc.dram_tensor(
        "sumexp", [P, TCOLS], mybir.dt.float32, kind="ExternalOutput"
    )

    # shard row n = b*P + p  ->  xb[b][p, c];  stat column = b
    xb = x.ap().rearrange("(b p) c -> b p c", p=P)

    # DMA tile sizes in 128-row blocks: small ramp-up (first ACTIVATE can
    # start ~4us in instead of waiting for a full 2MB load), steady 4-block
    # state (DMA efficiency), small ramp-down (short pipeline tail).
    SIZES = [1, 1, 2, 4] + [4] * 13 + [2, 1, 1]
    assert sum(SIZES) == TCOLS
    # (tile_idx, block_offset) 4-block groups whose row-sums go to DVE.
    # Measured: offloading sums to DVE adds cross-engine coupling stalls
    # that outweigh the ACT savings — keep ACT as the single pacer.
    DVE_SUM_GROUPS = set()

    with tile.TileContext(nc) as tc:
        with ExitStack() as ctx:
            xin = ctx.enter_context(tc.tile_pool(name="xin", bufs=6))
            expp = ctx.enter_context(tc.tile_pool(name="expp", bufs=2))
            scrp = ctx.enter_context(tc.tile_pool(name="scrp", bufs=1))
            stats = ctx.enter_context(tc.tile_pool(name="stats", bufs=1))

            smax = stats.tile([P, TCOLS], mybir.dt.float32)
            ssum = stats.tile([P, TCOLS], mybir.dt.float32)
            # garbage sink for the elementwise output of fused ACT accum
            ascr = scrp.tile([P, C], mybir.dt.float32)

            b0 = 0
            for ti, nb in enumerate(SIZES):
                xt = xin.tile([P, nb, C], mybir.dt.float32)
                nc.sync.dma_start(
                    out=xt, in_=xb[b0 : b0 + nb].rearrange("b p c -> p b c")
                )

                # row-max on DVE: one tensor_reduce per DMA tile
                nc.vector.reduce_max(
                    smax[:, b0 : b0 + nb], xt, axis=mybir.AxisListType.X
                )

                i = 0
                while i < nb:
                    if (ti, i) in DVE_SUM_GROUPS:
                        # 4-block group: one big exp on ACT, row-sums on DVE
                        et = expp.tile([P, 4, C], mybir.dt.float32)
                        nc.scalar.activation(
                            et, xt[:, i : i + 4, :],
                            mybir.ActivationFunctionType.Exp,
                        )
                        nc.vector.reduce_sum(
                            ssum[:, b0 + i : b0 + i + 4],
                            et,
                            axis=mybir.AxisListType.X,
                        )
                        i += 4
                    else:
                        col = b0 + i
                        nc.scalar.activation(
                            ascr,
                            xt[:, i, :],
                            mybir.ActivationFunctionType.Exp,
                            accum_out=ssum[:, col : col + 1],
                        )
                        i += 1
                b0 += nb

            nc.sync.dma_start(out=rowmax.ap(), in_=smax)
            nc.sync.dma_start(out=sumexp.ap(), in_=ssum)
    nc.compile()
    return nc


def _run_spmd(output_f32, trace=False, **kwargs):
    """Run the streaming pass on 8 cores. Returns (rowmax[N], sumexp[N], results)."""
    global _NC_CACHE
    if _NC_CACHE is None:
        _NC_CACHE = _build_bass()
    nc = _NC_CACHE
    in_maps = [
        {"x": output_f32[c * NSHARD : (c + 1) * NSHARD]} for c in range(N_CORES)
    ]
    res = run_bass_kernel_spmd(
        nc, in_maps, core_ids=list(range(N_CORES)), trace=trace, **kwargs
    )
    # out[p, col] -> shard row col*128 + p
    rowmax = np.concatenate(
        [r["rowmax"].T.reshape(-1) for r in res.results]
    )
    sumexp = np.concatenate(
        [r["sumexp"].T.reshape(-1) for r in res.results]
    )
    return rowmax, sumexp, res


def kernel(output, target, optimal_uncertainty_threshold, num_classes):
    output = np.ascontiguousarray(np.asarray(output), dtype=np.float32)
    target = np.asarray(target).astype(np.int64)
    th = float(np.asarray(optimal_uncertainty_threshold).reshape(-1)[0])
    c = float(int(num_classes))

    last_err = None
    for attempt in range(3):
        try:
            rowmax, sumexp, _ = _run_spmd(output)
            break
        except Exception as e:  # transient NRT_EXEC_UNIT_UNRECOVERABLE etc.
            last_err = e
            import time

            time.sleep(15 * (attempt + 1))
    else:
        raise last_err

    max_alpha = np.exp(rowmax.astype(np.float64)) + 1.0
    sum_alpha = sumexp.astype(np.float64) + c
    unc = c / sum_alpha

    umin = unc.min()
    umax = unc.max()
    unc_th = umin + th * (umax - umin)

    picked = output[np.arange(N), target]
    correct = picked == rowmax
    certain = unc <= unc_th
    t = np.tanh(unc)

    n_ac = np.sum(np.where(correct & certain, max_alpha * (1.0 - t), 0.0))
    n_au = np.sum(np.where(correct & ~certain, max_alpha * t, 0.0))
    n_ic = np.sum(np.where(~correct & certain, (1.0 - max_alpha) * (1.0 - t), 0.0))
    n_iu = np.sum(np.where(~correct & ~certain, (1.0 - max_alpha) * t, 0.0))

    evu = (n_ac + n_iu) / (n_ac + n_au + n_ic + n_iu + EPS)
    loss = -BETA * np.log(evu + EPS)
    return np.array([loss], dtype=np.float32)


# revision 21
# speedup vs baseline: 1.4273x; 1.0098x over previous
"""EvULoss Trainium2 kernel.

Math (see the EvU loss definition):
    evidence = exp(logits); alpha = evidence + 1
    max_alpha  = exp(rowmax(logits)) + 1
    predictions== target  <=>  logits[n, target[n]] == rowmax(logits[n])   (tie-free)
    sum_alpha  = rowsum(exp(logits)) + C
    unc        = C / sum_alpha
Everything downstream of {rowmax, rowsumexp} is O(N) and runs at gather
time on the host; the O(N*C) streaming pass (DMA + exp + reductions) runs
on 8 NeuronCores, data-parallel over N.

Per core: shard [8192, 1000] f32 as 64 blocks of [128 rows, 1000], DMA'd in
ramped tiles ([1,1,2,4]+[4]*13+[2,1,1] blocks; 0.5-2 MB HWDGE transfers).
  - VectorE: reduce_max over free dims -> rowmax columns (one op per tile)
  - ScalarE: exp with accum_out       -> rowsumexp column per 128-row block
    (the fused accumulator read pipelines with the next ACTIVATE; ScalarE is
    the single pacing engine at ~92% busy — measured faster than splitting
    work across engines, which adds coupling stalls)
Outputs per core: rowmax[128, 64], sumexp[128, 64] with shard row = col*128+p.
"""

from contextlib import ExitStack

import numpy as np

import concourse.bacc as bacc
import concourse.tile as tile
from concourse import mybir
from concourse.bass_utils import run_bass_kernel_spmd

N_CORES = 8
N, C = 65536, 1000
NSHARD = N // N_CORES          # 8192 rows per core
P = 128                        # SBUF partitions
BLK = 4                        # 128-row blocks per DMA: [128, 4, 1000] f32 = 2 MB
NT = NSHARD // (P * BLK)       # 16 outer iterations
TCOLS = NSHARD // P            # 64 stat columns
EPS = 1e-10
BETA = 1.0

_NC_CACHE = None


def _build_bass():
    nc = bacc.Bacc("TRN2", target_bir_lowering=False)
    x = nc.dram_tensor("x", [NSHARD, C], mybir.dt.float32, kind="ExternalInput")
    rowmax = nc.dram_tensor(
        "rowmax", [P, TCOLS], mybir.dt.float32, kind="ExternalOutput"
    )
    sumexp = nc.dram_tensor(
        "sumexp", [P, TCOLS], mybir.dt.float32, kind="ExternalOutput"
    )
    sumexp2 = nc.dram_tensor(
        "sumexp2", [P, TCOLS], mybir.dt.float32, kind="ExternalOutput"
    )

    # shard row n = b*P + p  ->  xb[b][p, c];  stat column = b
    xb = x.ap().rearrange("(b p) c -> b p c", p=P)

    # DMA tile sizes in 128-row blocks: small ramp-up (first ACTIVATE can
    # start ~4us in instead of waiting for a full 2MB load), steady 4-block
    # state (DMA efficiency), small ramp-down (short pipeline tail).
    SIZES = [1, 1, 2, 4] + [4] * 13 + [2, 1, 1]
    assert sum(SIZES) == TCOLS
    # (tile_idx, block_offset) 4-block groups whose row-sums go to DVE.
    # DVE writes its sums to a SEPARATE stats tile (ssum2) — sharing one
    # tile between ACT and DVE writers serializes the engines (measured:
    # +12us of ACT stalls). Host merges the disjoint columns by addition.
    DVE_SUM_GROUPS = {(6, 0), (10, 0), (14, 0)}

    with tile.TileContext(nc) as tc:
        with ExitStack() as ctx:
            xin = ctx.enter_context(tc.tile_pool(name="xin", bufs=6))
            expp = ctx.enter_context(tc.tile_pool(name="expp", bufs=2))
            scrp = ctx.enter_context(tc.tile_pool(name="scrp", bufs=1))
            stats = ctx.enter_context(tc.tile_pool(name="stats", bufs=1))

            smax = stats.tile([P, TCOLS], mybir.dt.float32)
            ssum = stats.tile([P, TCOLS], mybir.dt.float32)
            ssum2 = stats.tile([P, TCOLS], mybir.dt.float32)
            # garbage sink for the elementwise output of fused ACT accum
            ascr = scrp.tile([P, C], mybir.dt.float32)
            # each engine leaves its unowned columns at 0 for the host merge
            nc.gpsimd.memset(ssum, 0.0)
            nc.gpsimd.memset(ssum2, 0.0)

            b0 = 0
            for ti, nb in enumerate(SIZES):
                xt = xin.tile([P, nb, C], mybir.dt.float32)
                nc.sync.dma_start(
                    out=xt, in_=xb[b0 : b0 + nb].rearrange("b p c -> p b c")
                )

                # row-max on DVE: one tensor_reduce per DMA tile
                nc.vector.reduce_max(
                    smax[:, b0 : b0 + nb], xt, axis=mybir.AxisListType.X
                )

                i = 0
                while i < nb:
                    if (ti, i) in DVE_SUM_GROUPS:
                        # 4-block group: one big exp on ACT, row-sums on DVE
                        et = expp.tile([P, 4, C], mybir.dt.float32)
                        nc.scalar.activation(
                            et, xt[:, i : i + 4, :],
                            mybir.ActivationFunctionType.Exp,
                        )
                        nc.vector.reduce_sum(
                            ssum2[:, b0 + i : b0 + i + 4],
                            et,
                            axis=mybir.AxisListType.X,
                        )
                        i += 4
                    else:
                        col = b0 + i
                        nc.scalar.activation(
                            ascr,
                            xt[:, i, :],
                            mybir.ActivationFunctionType.Exp,
                            accum_out=ssum[:, col : col + 1],
                        )
                        i += 1
                b0 += nb

            nc.sync.dma_start(out=rowmax.ap(), in_=smax)
            nc.sync.dma_start(out=sumexp.ap(), in_=ssum)
            nc.sync.dma_start(out=sumexp2.ap(), in_=ssum2)
    nc.compile()
    return nc


def _run_spmd(output_f32, trace=False, **kwargs):
    """Run the streaming pass on 8 cores. Returns (rowmax[N], sumexp[N], results)."""
    global _NC_CACHE
    if _NC_CACHE is None:
        _NC_CACHE = _build_bass()
    nc = _NC_CACHE
    in_maps = [
        {"x": output_f32[c * NSHARD : (c + 1) * NSHARD]} for c in range(N_CORES)
    ]
    res = run_bass_kernel_spmd(
        nc, in_maps, core_ids=list(range(N_CORES)), trace=trace, **kwargs
    )
    # out[p, col] -> shard row col*128 + p
    rowmax = np.concatenate(
        [r["rowmax"].T.reshape(-1) for r in res.results]
    )
    sumexp = np.concatenate(
        [(r["sumexp"] + r["sumexp2"]).T.reshape(-1) for r in res.results]
    )
    return rowmax, sumexp, res


def kernel(output, target, optimal_uncertainty_threshold, num_classes):
    output = np.ascontiguousarray(np.asarray(output), dtype=np.float32)
    target = np.asarray(target).astype(np.int64)
    th = float(np.asarray(optimal_uncertainty_threshold).reshape(-1)[0])
    c = float(int(num_classes))

    last_err = None
    for attempt in range(3):
        try:
            rowmax, sumexp, _ = _run_spmd(output)
            break
        except Exception as e:  # transient NRT_EXEC_UNIT_UNRECOVERABLE etc.
            last_err = e
            import time

            time.sleep(15 * (attempt + 1))
    else:
        raise last_err

    max_alpha = np.exp(rowmax.astype(np.float64)) + 1.0
    sum_alpha = sumexp.astype(np.float64) + c
    unc = c / sum_alpha

    umin = unc.min()
    umax = unc.max()
    unc_th = umin + th * (umax - umin)

    picked = output[np.arange(N), target]
    correct = picked == rowmax
    certain = unc <= unc_th
    t = np.tanh(unc)

    n_ac = np.sum(np.where(correct & certain, max_alpha * (1.0 - t), 0.0))
    n_au = np.sum(np.where(correct & ~certain, max_alpha * t, 0.0))
    n_ic = np.sum(np.where(~correct & certain, (1.0 - max_alpha) * (1.0 - t), 0.0))
    n_iu = np.sum(np.where(~correct & ~certain, (1.0 - max_alpha) * t, 0.0))

    evu = (n_ac + n_iu) / (n_ac + n_au + n_ic + n_iu + EPS)
    loss = -BETA * np.log(evu + EPS)
    return np.array([loss], dtype=np.float32)
